# revision 4
# baseline (speedup 1.0000x reference)
"""AUCLoss kernel for 8 TRN2 NeuronCores.

Math: loss = sum_{i,j} pw_i * nw_j * softplus(p_j - p_i) / (n_pos * n_neg)
where pw/nw are per-element weights masked to label==1 / label==0.

Instead of materializing the N x N (13184^2) pairwise matrix, we use a
truncated Fourier expansion of softplus on the diff domain [-12, 12]:

    softplus(x) ~= sum_k a_k cos(w_k x) + b_k sin(w_k x),   w_k = k*pi/16

which separates over pairs (x = n - p):
    cos(w(n-p)) = cos(wn)cos(wp) + sin(wn)sin(wp)
    sin(w(n-p)) = sin(wn)cos(wp) - cos(wn)sin(wp)

So the whole pairwise sum reduces to weighted Fourier feature sums
    C[w,k] = sum_i w_i cos(w_k p_i),  S[w,k] = sum_i w_i sin(w_k p_i)
for the four weight vectors w in {pw, nw, mask1, mask0}, then a tiny
bilinear combine. Max abs fit error ~5e-6 -> final rel err ~2e-6.

Each core processes 1648 points (13 blocks of 128):
  - one matmul against a constant block-diagonal frequency matrix builds
    all phase arguments w_k * x + phi (phi = pi/2 turns sin into cos)
  - one Sin activation produces all 2F=64 features per point
  - 4 DVE ops build [pw, nw, mask1, mask0] from labels/weights
  - 13 accumulating matmuls contract points -> [4, 64] partial sums
Host sums the 8 per-core [4, 64] partials and applies the combine.
"""

import numpy as np

import concourse.bass as bass
import concourse.mybir as mybir
from concourse.tile import TileContext
from concourse.bass_utils import run_bass_kernel_spmd

# ---------------------------------------------------------------- constants
B, C = 64, 206
N = B * C                      # 13184 flattened preds
NCORES = 8
CHUNK = N // NCORES            # 1648 points per core
BLOCKS = 13                    # ceil(1648 / 128)
PAD = BLOCKS * 128             # 1664 (16 zero-pad points per core)
F = 32                         # frequencies
NFEAT = 2 * F                  # 64 features per point (cos block | sin block)
PERIOD = 16.0                  # half-period of the Fourier basis
FIT_X = 12.0                   # fit domain for softplus diffs

_f32 = mybir.dt.float32


def _fit_fourier():
    """Least-squares fit softplus(x) on [-FIT_X, FIT_X] in the basis
    {cos(w_k x), sin(w_k x)}. Deterministic, ~ms of numpy."""
    w = np.arange(F) * np.pi / PERIOD
    xs = np.linspace(-FIT_X, FIT_X, 8001)
    A = np.concatenate(
        [np.cos(np.outer(xs, w)), np.sin(np.outer(xs, w))], axis=1
    )
    y = np.log1p(np.exp(-np.abs(xs))) + np.maximum(xs, 0.0)
    coef = np.linalg.solve(A.T @ A + 1e-9 * np.eye(2 * F), A.T @ y)
    return w, coef[:F], coef[F:]


_OMEGA, _COEF_A, _COEF_B = _fit_fourier()


def _mfeat_const():
    """[14, BLOCKS*NFEAT] block-diagonal frequency matrix + phase row.

    Column b*NFEAT + j: frequency _OMEGA[j % F] in row b (block-diagonal),
    phase pi/2 in row 13 for j < F (cos features), 0 for j >= F (sin).
    matmul(lhsT=[x blocks; ones], rhs=this) -> arg[p, (b,j)] = w_j*x_{b,p} + phi_j
    """
    m = np.zeros((BLOCKS + 1, BLOCKS * NFEAT), dtype=np.float32)
    for b in range(BLOCKS):
        m[b, b * NFEAT : b * NFEAT + F] = _OMEGA
        m[b, b * NFEAT + F : (b + 1) * NFEAT] = _OMEGA
        m[BLOCKS, b * NFEAT : b * NFEAT + F] = np.pi / 2
    return m


_NC_CACHE = None


def _build_nc():
    # Raw Bass (no TileContext): the pipeline is a short linear chain and
    # explicit semaphores avoid both the per-instruction sync-wait slot
    # limits and Tile's multi-microsecond kernel-tail drain/barrier.
    nc = bass.Bass()
    ncols = BLOCKS * NFEAT  # 832
    # xm: cols 0:128 = x blocks + ones row, cols 128: = frequency matrix
    xm = nc.declare_dram_parameter("xm", [BLOCKS + 1, 128 + ncols], _f32, isOutput=False)
    # lw: cols 0:13 = labels (transposed layout), 13:26 = element weights
    lw = nc.declare_dram_parameter("lw", [128, 2 * BLOCKS], _f32, isOutput=False)
    out = nc.declare_dram_parameter("out", [4, NFEAT], _f32, isOutput=True)

    eq = mybir.AluOpType.is_equal
    mult = mybir.AluOpType.mult

    with (
        nc.sbuf_tensor([BLOCKS + 1, 128 + ncols], _f32) as xm_t,
        nc.sbuf_tensor([128, 2 * BLOCKS], _f32) as lw_t,
        nc.sbuf_tensor([128, ncols], _f32) as feat,
        nc.sbuf_tensor([128, 4 * BLOCKS], _f32) as w4,
        nc.sbuf_tensor([4, NFEAT], _f32) as out_t,
        nc.psum_tensor([128, ncols], _f32) as arg,
        nc.psum_tensor([4, NFEAT], _f32) as red,
        nc.semaphore() as dma_sem,
        nc.semaphore() as s_pe,
        nc.semaphore() as s_act,
        nc.semaphore() as s_dve,
        nc.semaphore() as s_done,
        nc.Block() as block,
    ):

        @block.sync
        def _(sync):
            sync.dma_start(out=xm_t[:], in_=xm[:]).then_inc(dma_sem, 16)
            sync.dma_start(out=lw_t[:], in_=lw[:]).then_inc(dma_sem, 16)
            sync.wait_ge(s_done, 1)
            sync.dma_start(out=out[:], in_=out_t[:]).then_inc(dma_sem, 16)

        @block.tensor
        def _(tensor):
            tensor.wait_ge(dma_sem, 32)
            # phase arguments: arg[p, (b,j)] = w_j * x_{b,p} + phi_j
            tensor.matmul(
                arg[:, 0:512],
                xm_t[:, 0:128],
                xm_t[:, 128 : 128 + 512],
                start=True,
                stop=True,
            )
            tensor.matmul(
                arg[:, 512:ncols],
                xm_t[:, 0:128],
                xm_t[:, 128 + 512 : 128 + ncols],
                start=True,
                stop=True,
            ).then_inc(s_pe, 1)
            tensor.wait_ge(s_act, 1)
            tensor.wait_ge(s_dve, 1)
            # contract points: red[w, j] += sum_p w4[p, w*13+b] * feat[p, b*64+j]
            for b in range(BLOCKS):
                mm = tensor.matmul(
                    red[:],
                    w4[:, b : b + 3 * BLOCKS + 1 : BLOCKS],  # cols b, b+13, b+26, b+39
                    feat[:, b * NFEAT : (b + 1) * NFEAT],
                    start=(b == 0),
                    stop=(b == BLOCKS - 1),
                )
            mm.then_inc(s_pe, 1)

        @block.scalar
        def _(scalar):
            scalar.wait_ge(s_pe, 1)
            # all 64 Fourier features for all 1664 points in one activation
            scalar.activation(
                feat[:], arg[:], mybir.ActivationFunctionType.Sin
            ).then_inc(s_act, 1)

        @block.vector
        def _(vector):
            vector.wait_ge(dma_sem, 32)
            lab_t = lw_t[:, 0:BLOCKS]
            w_t = lw_t[:, BLOCKS : 2 * BLOCKS]
            # weight columns: [pw | nw | mask1 | mask0], each [128, BLOCKS]
            vector.scalar_tensor_tensor(
                w4[:, 0:BLOCKS], lab_t, 1.0, w_t, op0=eq, op1=mult
            )
            vector.scalar_tensor_tensor(
                w4[:, BLOCKS : 2 * BLOCKS], lab_t, 0.0, w_t, op0=eq, op1=mult
            )
            vector.tensor_scalar(
                w4[:, 2 * BLOCKS : 3 * BLOCKS], lab_t, 1.0, None, op0=eq
            )
            vector.tensor_scalar(
                w4[:, 3 * BLOCKS : 4 * BLOCKS], lab_t, 0.0, None, op0=eq
            ).then_inc(s_dve, 1)
            vector.wait_ge(s_pe, 2)
            vector.tensor_copy(out_t[:], red[:]).then_inc(s_done, 1)

    return nc


def _shard_inputs(preds, sample_weights, labels):
    """Build per-core input maps. Layout transforms only (no math beyond
    the f32 cast of labels)."""
    p = np.ascontiguousarray(preds, dtype=np.float32).reshape(-1)
    lab = np.ascontiguousarray(labels).reshape(-1).astype(np.float32)
    wfull = np.repeat(
        np.ascontiguousarray(sample_weights, dtype=np.float32), C
    )  # per-element sample weight
    mf = _mfeat_const()

    in_maps = []
    for c in range(NCORES):
        sl = slice(c * CHUNK, (c + 1) * CHUNK)
        xpad = np.zeros(PAD, dtype=np.float32)
        xpad[:CHUNK] = p[sl]
        xm = np.zeros((BLOCKS + 1, 128 + BLOCKS * NFEAT), dtype=np.float32)
        xm[:BLOCKS, :128] = xpad.reshape(BLOCKS, 128)
        xm[BLOCKS, :128] = 1.0  # ones row -> phase offsets
        xm[:, 128:] = mf

        lpad = np.full(PAD, -1.0, dtype=np.float32)  # pad label -1: not pos/neg
        lpad[:CHUNK] = lab[sl]
        wpad = np.zeros(PAD, dtype=np.float32)
        wpad[:CHUNK] = wfull[sl]
        lwm = np.concatenate(
            [lpad.reshape(BLOCKS, 128).T, wpad.reshape(BLOCKS, 128).T], axis=1
        )

        in_maps.append({"xm": xm, "lw": np.ascontiguousarray(lwm)})
    return in_maps


def _combine(partials):
    """Sum per-core [4, 64] feature sums and apply the bilinear combine."""
    s = np.zeros((4, NFEAT), dtype=np.float64)
    for part in partials:
        s += part.astype(np.float64)
    cp, sp = s[0, :F], s[0, F:]        # pos-weighted cos/sin sums
    cn, sn = s[1, :F], s[1, F:]        # neg-weighted cos/sin sums
    n_pos = s[2, 0]                    # mask1 . cos(0*x) = count(label==1)
    n_neg = s[3, 0]                    # mask0 . cos(0*x) = count(label==0)
    total = np.sum(
        _COEF_A * (cn * cp + sn * sp) + _COEF_B * (sn * cp - cn * sp)
    )
    return np.asarray(total / (n_pos * n_neg), dtype=np.float32)


def run_on_device(preds, sample_weights, labels, trace=False, **spmd_kwargs):
    """Shard, run the SPMD kernel on cores 0-7, return (result, BassKernelResults)."""
    global _NC_CACHE
    if _NC_CACHE is None:
        _NC_CACHE = _build_nc()
    in_maps = _shard_inputs(preds, sample_weights, labels)
    res = run_bass_kernel_spmd(
        _NC_CACHE, in_maps, core_ids=list(range(NCORES)), trace=trace, **spmd_kwargs
    )
    partials = [res.results[i]["out"] for i in range(NCORES)]
    return _combine(partials), res


def kernel(preds, sample_weights, labels):
    result, _ = run_on_device(preds, sample_weights, labels)
    return result


# revision 5
# speedup vs baseline: 1.4444x; 1.4444x over previous
"""AUCLoss kernel for 8 TRN2 NeuronCores.

Math: loss = sum_{i,j} pw_i * nw_j * softplus(p_j - p_i) / (n_pos * n_neg)
where pw/nw are per-element weights masked to label==1 / label==0.

Instead of materializing the N x N (13184^2) pairwise matrix, we use a
truncated Fourier expansion of softplus on the diff domain [-12, 12]:

    softplus(x) ~= sum_k a_k cos(w_k x) + b_k sin(w_k x),   w_k = k*pi/16

which separates over pairs (x = n - p):
    cos(w(n-p)) = cos(wn)cos(wp) + sin(wn)sin(wp)
    sin(w(n-p)) = sin(wn)cos(wp) - cos(wn)sin(wp)

So the whole pairwise sum reduces to weighted Fourier feature sums
    C[w,k] = sum_i w_i cos(w_k p_i),  S[w,k] = sum_i w_i sin(w_k p_i)
for the four weight vectors w in {pw, nw, mask1, mask0}, then a tiny
bilinear combine. Max abs fit error ~5e-6 -> final rel err ~2e-6.

Each core processes 1648 points (13 blocks of 128):
  - one matmul against a constant block-diagonal frequency matrix builds
    all phase arguments w_k * x + phi (phi = pi/2 turns sin into cos)
  - one Sin activation produces all 2F=64 features per point
  - 4 DVE ops build [pw, nw, mask1, mask0] from labels/weights
  - 13 accumulating matmuls contract points -> [4, 64] partial sums
Host sums the 8 per-core [4, 64] partials and applies the combine.
"""

import numpy as np

import concourse.bass as bass
import concourse.mybir as mybir
from concourse.tile import TileContext
from concourse.bass_utils import run_bass_kernel_spmd

# ---------------------------------------------------------------- constants
B, C = 64, 206
N = B * C                      # 13184 flattened preds
NCORES = 8
CHUNK = N // NCORES            # 1648 points per core
BLOCKS = 13                    # ceil(1648 / 128)
PAD = BLOCKS * 128             # 1664 (16 zero-pad points per core)
F = 32                         # frequencies
NFEAT = 2 * F                  # 64 features per point (cos block | sin block)
PERIOD = 16.0                  # half-period of the Fourier basis
FIT_X = 12.0                   # fit domain for softplus diffs

_f32 = mybir.dt.float32


def _fit_fourier():
    """Least-squares fit softplus(x) on [-FIT_X, FIT_X] in the basis
    {cos(w_k x), sin(w_k x)}. Deterministic, ~ms of numpy."""
    w = np.arange(F) * np.pi / PERIOD
    xs = np.linspace(-FIT_X, FIT_X, 8001)
    A = np.concatenate(
        [np.cos(np.outer(xs, w)), np.sin(np.outer(xs, w))], axis=1
    )
    y = np.log1p(np.exp(-np.abs(xs))) + np.maximum(xs, 0.0)
    coef = np.linalg.solve(A.T @ A + 1e-9 * np.eye(2 * F), A.T @ y)
    return w, coef[:F], coef[F:]


_OMEGA, _COEF_A, _COEF_B = _fit_fourier()


def _mfeat_const():
    """[14, BLOCKS*NFEAT] block-diagonal frequency matrix + phase row.

    Column b*NFEAT + j: frequency _OMEGA[j % F] in row b (block-diagonal),
    phase pi/2 in row 13 for j < F (cos features), 0 for j >= F (sin).
    matmul(lhsT=[x blocks; ones], rhs=this) -> arg[p, (b,j)] = w_j*x_{b,p} + phi_j
    """
    m = np.zeros((BLOCKS + 1, BLOCKS * NFEAT), dtype=np.float32)
    for b in range(BLOCKS):
        m[b, b * NFEAT : b * NFEAT + F] = _OMEGA
        m[b, b * NFEAT + F : (b + 1) * NFEAT] = _OMEGA
        m[BLOCKS, b * NFEAT : b * NFEAT + F] = np.pi / 2
    return m


_NC_CACHE = None


def _build_nc():
    # Raw Bass (no TileContext): the pipeline is a short linear chain and
    # explicit semaphores avoid both the per-instruction sync-wait slot
    # limits and Tile's multi-microsecond kernel-tail drain/barrier.
    #
    # Scheduling notes:
    # - a dummy Sin on ScalarE before any waits pulls the ~2.7us ACT
    #   table load off the critical path (overlaps preamble + DMA)
    # - the Sin activation is split in two so the second half overlaps
    #   the first reduction matmuls
    # - features and weight masks are bf16: reduction matmuls then run
    #   single-pass instead of fp32's LOW/HIGH double pass (costs ~1e-4
    #   relative error, far inside tolerance); the phase-argument matmul
    #   stays fp32 (phases up to ~25 rad need fp32 precision)
    # - the two input DMAs go on different queues (SP + ACT engines) with
    #   separate semaphores; PE only waits for its own input
    nc = bass.Bass(enable_partition_id=False, monotonic_sem_count=0)
    ncols = BLOCKS * NFEAT  # 832
    # xm: cols 0:128 = x blocks + ones row, cols 128: = frequency matrix
    xm = nc.declare_dram_parameter("xm", [BLOCKS + 1, 128 + ncols], _f32, isOutput=False)
    # lw: cols 0:13 = labels (transposed layout), 13:26 = element weights
    lw = nc.declare_dram_parameter("lw", [128, 2 * BLOCKS], _f32, isOutput=False)
    out = nc.declare_dram_parameter("out", [4, NFEAT], _f32, isOutput=True)

    eq = mybir.AluOpType.is_equal
    mult = mybir.AluOpType.mult
    _bf16 = mybir.dt.bfloat16
    sin_f = mybir.ActivationFunctionType.Sin

    with (
        nc.sbuf_tensor([BLOCKS + 1, 128 + ncols], _f32) as xm_t,
        nc.sbuf_tensor([128, 2 * BLOCKS], _f32) as lw_t,
        nc.sbuf_tensor([128, ncols], _bf16) as feat,
        nc.sbuf_tensor([128, 4 * BLOCKS], _bf16) as w4,
        nc.sbuf_tensor([128, 1], _f32) as scratch,
        nc.sbuf_tensor([4, NFEAT], _f32) as out_t,
        nc.psum_tensor([128, ncols], _f32) as arg,
        nc.psum_tensor([4, NFEAT], _f32) as red,
        nc.semaphore() as dma_x,
        nc.semaphore() as dma_l,
        nc.semaphore() as s_pe,
        nc.semaphore() as s_act,
        nc.semaphore() as s_dve,
        nc.semaphore() as s_done,
        nc.Block() as block,
    ):
        zero_ap = nc.const_aps.aps[(mybir.dt.float32, 0.0)]

        @block.sync
        def _(sync):
            sync.dma_start(out=xm_t[:], in_=xm[:]).then_inc(dma_x, 16)
            sync.wait_ge(s_done, 1)
            sync.dma_start(out=out[:], in_=out_t[:]).then_inc(dma_x, 16)

        @block.scalar
        def _(scalar):
            scalar.dma_start(out=lw_t[:], in_=lw[:]).then_inc(dma_l, 16)
            # dummy: forces the Sin table load early, off the critical path
            scalar.activation(scratch[:], zero_ap, sin_f)
            scalar.wait_ge(s_pe, 1)
            scalar.activation(feat[:, 0:512], arg[:, 0:512], sin_f).then_inc(s_act, 1)
            scalar.wait_ge(s_pe, 2)
            scalar.activation(feat[:, 512:ncols], arg[:, 512:ncols], sin_f).then_inc(
                s_act, 1
            )

        @block.tensor
        def _(tensor):
            tensor.wait_ge(dma_x, 16)
            # phase arguments: arg[p, (b,j)] = w_j * x_{b,p} + phi_j
            tensor.matmul(
                arg[:, 0:512],
                xm_t[:, 0:128],
                xm_t[:, 128 : 128 + 512],
                start=True,
                stop=True,
            ).then_inc(s_pe, 1)
            tensor.matmul(
                arg[:, 512:ncols],
                xm_t[:, 0:128],
                xm_t[:, 128 + 512 : 128 + ncols],
                start=True,
                stop=True,
            ).then_inc(s_pe, 1)
            tensor.wait_ge(s_act, 1)
            tensor.wait_ge(s_dve, 1)
            # contract points: red[w, j] += sum_p w4[p, w*13+b] * feat[p, b*64+j]
            for b in range(8):
                tensor.matmul(
                    red[:],
                    w4[:, b : b + 3 * BLOCKS + 1 : BLOCKS],  # cols b, b+13, b+26, b+39
                    feat[:, b * NFEAT : (b + 1) * NFEAT],
                    start=(b == 0),
                    stop=False,
                )
            tensor.wait_ge(s_act, 2)
            for b in range(8, BLOCKS):
                mm = tensor.matmul(
                    red[:],
                    w4[:, b : b + 3 * BLOCKS + 1 : BLOCKS],
                    feat[:, b * NFEAT : (b + 1) * NFEAT],
                    start=False,
                    stop=(b == BLOCKS - 1),
                )
            mm.then_inc(s_pe, 1)

        @block.vector
        def _(vector):
            vector.wait_ge(dma_l, 16)
            lab_t = lw_t[:, 0:BLOCKS]
            w_t = lw_t[:, BLOCKS : 2 * BLOCKS]
            # weight columns: [pw | nw | mask1 | mask0], each [128, BLOCKS]
            vector.scalar_tensor_tensor(
                w4[:, 0:BLOCKS], lab_t, 1.0, w_t, op0=eq, op1=mult
            )
            vector.scalar_tensor_tensor(
                w4[:, BLOCKS : 2 * BLOCKS], lab_t, 0.0, w_t, op0=eq, op1=mult
            )
            vector.tensor_scalar(
                w4[:, 2 * BLOCKS : 3 * BLOCKS], lab_t, 1.0, None, op0=eq
            )
            vector.tensor_scalar(
                w4[:, 3 * BLOCKS : 4 * BLOCKS], lab_t, 0.0, None, op0=eq
            ).then_inc(s_dve, 1)
            vector.wait_ge(s_pe, 3)
            vector.tensor_copy(out_t[:], red[:]).then_inc(s_done, 1)

    return nc


def _shard_inputs(preds, sample_weights, labels):
    """Build per-core input maps. Layout transforms only (no math beyond
    the f32 cast of labels)."""
    p = np.ascontiguousarray(preds, dtype=np.float32).reshape(-1)
    lab = np.ascontiguousarray(labels).reshape(-1).astype(np.float32)
    wfull = np.repeat(
        np.ascontiguousarray(sample_weights, dtype=np.float32), C
    )  # per-element sample weight
    mf = _mfeat_const()

    in_maps = []
    for c in range(NCORES):
        sl = slice(c * CHUNK, (c + 1) * CHUNK)
        xpad = np.zeros(PAD, dtype=np.float32)
        xpad[:CHUNK] = p[sl]
        xm = np.zeros((BLOCKS + 1, 128 + BLOCKS * NFEAT), dtype=np.float32)
        xm[:BLOCKS, :128] = xpad.reshape(BLOCKS, 128)
        xm[BLOCKS, :128] = 1.0  # ones row -> phase offsets
        xm[:, 128:] = mf

        lpad = np.full(PAD, -1.0, dtype=np.float32)  # pad label -1: not pos/neg
        lpad[:CHUNK] = lab[sl]
        wpad = np.zeros(PAD, dtype=np.float32)
        wpad[:CHUNK] = wfull[sl]
        lwm = np.concatenate(
            [lpad.reshape(BLOCKS, 128).T, wpad.reshape(BLOCKS, 128).T], axis=1
        )

        in_maps.append({"xm": xm, "lw": np.ascontiguousarray(lwm)})
    return in_maps


def _combine(partials):
    """Sum per-core [4, 64] feature sums and apply the bilinear combine."""
    s = np.zeros((4, NFEAT), dtype=np.float64)
    for part in partials:
        s += part.astype(np.float64)
    cp, sp = s[0, :F], s[0, F:]        # pos-weighted cos/sin sums
    cn, sn = s[1, :F], s[1, F:]        # neg-weighted cos/sin sums
    n_pos = s[2, 0]                    # mask1 . cos(0*x) = count(label==1)
    n_neg = s[3, 0]                    # mask0 . cos(0*x) = count(label==0)
    total = np.sum(
        _COEF_A * (cn * cp + sn * sp) + _COEF_B * (sn * cp - cn * sp)
    )
    return np.asarray(total / (n_pos * n_neg), dtype=np.float32)


def run_on_device(preds, sample_weights, labels, trace=False, **spmd_kwargs):
    """Shard, run the SPMD kernel on cores 0-7, return (result, BassKernelResults)."""
    global _NC_CACHE
    if _NC_CACHE is None:
        _NC_CACHE = _build_nc()
    in_maps = _shard_inputs(preds, sample_weights, labels)
    res = run_bass_kernel_spmd(
        _NC_CACHE, in_maps, core_ids=list(range(NCORES)), trace=trace, **spmd_kwargs
    )
    partials = [res.results[i]["out"] for i in range(NCORES)]
    return _combine(partials), res


def kernel(preds, sample_weights, labels):
    result, _ = run_on_device(preds, sample_weights, labels)
    return result
